# revision 1
# baseline (speedup 1.0000x reference)
"""Causal self-attention (B=4, T=2048, D=1024, H=16) on 8 TRN2 NeuronCores.

Sharding: core c handles batch b=c//2 and head-group g=c%2 (8 heads).
Each core computes its heads' attention + a partial output projection
(contraction over its 512 attn channels); the host sums the two partials
per batch and adds b_out.

Per-core device kernel (all matmuls fp32r, transposed "channels on
partitions" layout):
  qk-proj   qkT[ch,T] = wqk.T @ xT          (ch-major, per head-pair m-chunks)
  v-proj    V[t,ch']  = xT.T @ wv_aug        (t-major, 65-wide per head: 64 v
                                              cols + a ones col for the softmax
                                              normalizer; bias via ones-row mm)
  rope      q',k' via DVE/gpsimd elementwise with host-built cos/sin tables
  S^T       [k,q] = k'^T q' per head, 2 heads packed in the PE array via
            tile_position row tiling (K=64 each)
  softmax   no-max-subtraction exp (score range validated ~|8|), causal mask
            added in PSUM on diagonal tiles, normalizer from the V ones col
  PV        attn_aug^T[65,q] = V_aug^T @ E^T accumulated over k blocks
  norm      attnT = attn_aug[0:64] * bcast(1/Z)
  out-proj  out[q,o] = attnT.T @ wo  (partial; host adds pair partials)
"""
import sys
import numpy as np

for _p in ("/opt/trn_rl_repo", "/root/.axon_site/_ro/trn_rl_repo"):
    if _p not in sys.path:
        sys.path.append(_p)

import concourse.bass as bass
import concourse.bacc as bacc
import concourse.tile as tile
import concourse.mybir as mybir
from concourse import bass_utils

F32 = mybir.dt.float32
F32R = mybir.dt.float32r
AF = mybir.ActivationFunctionType
ALU = mybir.AluOpType

B, T, D, H, DK = 4, 2048, 1024, 16, 64
NC_ = 8          # cores
HPG = 8          # heads per group
NPAIR = 4        # head pairs per core
KT = 8           # 128-row k-tiles over D
XC = 512         # x/qkv t-chunk width
NXC = T // XC    # 8
QC = 512         # attention q-chunk width
NQC = T // QC    # 4
NKB = T // 128   # 16 key blocks
MASK_VAL = -30000.0

_cache = {}


def _build_nc(trace_scopes=False):
    nc = bacc.Bacc("TRN2", target_bir_lowering=False, debug=False)

    xT_d = nc.dram_tensor("xT", [D, T], F32R, kind="ExternalInput").ap()
    wqk_d = nc.dram_tensor("wqk", [D, 1024], F32R, kind="ExternalInput").ap()
    wva_d = nc.dram_tensor("wva", [D, 520], F32R, kind="ExternalInput").ap()
    bva_d = nc.dram_tensor("bva", [1, 520], F32R, kind="ExternalInput").ap()
    ones_d = nc.dram_tensor("ones1", [1, 128], F32R, kind="ExternalInput").ap()
    wo_d = nc.dram_tensor("wo", [512, 1024], F32R, kind="ExternalInput").ap()
    bqk_d = nc.dram_tensor("bqk", [128, 8], F32, kind="ExternalInput").ap()
    cos_d = nc.dram_tensor("cos4", [128, T], F32, kind="ExternalInput").ap()
    sin_d = nc.dram_tensor("sin4", [128, T], F32, kind="ExternalInput").ap()
    out_d = nc.dram_tensor("out", [T, 1024], F32, kind="ExternalOutput").ap()

    with tile.TileContext(nc, pool_alloc_mode="queue") as tc:
        _emit(tc, nc, xT_d, wqk_d, wva_d, bva_d, ones_d, wo_d, bqk_d,
              cos_d, sin_d, out_d)
    nc.compile()
    return nc


def _emit(tc, nc, xT_d, wqk_d, wva_d, bva_d, ones_d, wo_d, bqk_d,
          cos_d, sin_d, out_d):
    from contextlib import ExitStack
    ctx = ExitStack()
    with ctx:
        consts = ctx.enter_context(tc.tile_pool(name="consts", bufs=1))
        vpool = ctx.enter_context(tc.tile_pool(name="vpool", bufs=1))
        qkp = ctx.enter_context(tc.tile_pool(name="qkp", bufs=8))
        ep = ctx.enter_context(tc.tile_pool(name="ep", bufs=5))
        zbp = ctx.enter_context(tc.tile_pool(name="zbp", bufs=2))
        atp = ctx.enter_context(tc.tile_pool(name="atp", bufs=16))
        ps_mm = ctx.enter_context(tc.tile_pool(name="ps_mm", bufs=2, space="PSUM"))
        ps_s = ctx.enter_context(tc.tile_pool(name="ps_s", bufs=2, space="PSUM"))
        ps_pv = ctx.enter_context(tc.tile_pool(name="ps_pv", bufs=2, space="PSUM"))

        # ---------------- constants ----------------
        cos_t = consts.tile([128, T], F32, tag="cos")
        nc.sync.dma_start(out=cos_t[:], in_=cos_d)
        sin_t = consts.tile([128, T], F32, tag="sin")
        nc.sync.dma_start(out=sin_t[:], in_=sin_d)
        bqk_t = consts.tile([128, 8], F32, tag="bqk")
        nc.sync.dma_start(out=bqk_t[:], in_=bqk_d)
        bva_t = consts.tile([1, 520], F32R, tag="bva")
        nc.sync.dma_start(out=bva_t[:], in_=bva_d)
        ones_t = consts.tile([1, 128], F32R, tag="ones")
        nc.sync.dma_start(out=ones_t[:], in_=ones_d)
        wva_t = consts.tile([128, KT, 520], F32R, tag="wva")
        nc.sync.dma_start(out=wva_t[:], in_=wva_d.rearrange("(k p) m -> p k m", p=128))
        # additive causal masks: tri block [128,128] (valid iff c-r>=0) and
        # the d=3 variant [128,256] = [all-masked | tri]
        mask_t = consts.tile([128, 128], F32, tag="mask")
        nc.gpsimd.memset(mask_t[:], 0.0)
        nc.gpsimd.affine_select(
            out=mask_t[:], in_=mask_t[:], compare_op=ALU.is_ge, fill=MASK_VAL,
            base=0, pattern=[[1, 128]], channel_multiplier=-1)
        mask3_t = consts.tile([128, 256], F32, tag="mask3")
        nc.gpsimd.memset(mask3_t[:, 0:128], MASK_VAL)
        nc.gpsimd.memset(mask3_t[:, 128:256], 0.0)
        nc.gpsimd.affine_select(
            out=mask3_t[:, 128:256], in_=mask3_t[:, 128:256], compare_op=ALU.is_ge,
            fill=MASK_VAL, base=0, pattern=[[1, 128]], channel_multiplier=-1)

        # V_aug for all 16 t-blocks: [128 tok, 16 * (8 heads * 65)]
        V_t = vpool.tile([128, NKB, 520], F32R, tag="V")

        xT_r = xT_d.rearrange("(k p) t -> p k t", p=128)
        wqk_r = wqk_d.rearrange("(k p) m -> p k m", p=128)

        at_tiles = []
        qkv_ctx = ExitStack()
        wqkp = qkv_ctx.enter_context(tc.tile_pool(name="wqkp", bufs=2))
        xp = qkv_ctx.enter_context(tc.tile_pool(name="xp", bufs=2))
        t1p = qkv_ctx.enter_context(tc.tile_pool(name="t1p", bufs=2))
        for p in range(NPAIR):
            # -------- load this pair's qk weight slice --------
            wqk_pair = wqkp.tile([128, KT, 256], F32R, tag="wqk")
            nc.sync.dma_start(out=wqk_pair[:], in_=wqk_r[:, :, 256 * p:256 * (p + 1)])

            qp_ts = [qkp.tile([128, QC], F32R, tag="qp", name=f"qp{p}_{i}") for i in range(NQC)]
            kp_ts = [qkp.tile([128, QC], F32R, tag="kp", name=f"kp{p}_{i}") for i in range(NQC)]

            for tq in range(NXC):
                c0 = tq * XC
                xc = xp.tile([128, KT, XC], F32R, tag="xc")
                nc.sync.dma_start(out=xc[:], in_=xT_r[:, :, c0:c0 + XC])

                if p == 0:
                    # ---- v-proj for the 2 t-blocks in this chunk ----
                    for tb2 in range(XC // 128):
                        tb = tq * (XC // 128) + tb2
                        for half in range(2):
                            h0 = half * 260
                            pvm = ps_mm.tile([128, 260], F32, tag="mm")
                            for k in range(KT):
                                nc.tensor.matmul(
                                    pvm[:], lhsT=xc[:, k, tb2 * 128:(tb2 + 1) * 128],
                                    rhs=wva_t[:, k, h0:h0 + 260],
                                    start=(k == 0), stop=False)
                            nc.tensor.matmul(pvm[:], lhsT=ones_t[:],
                                             rhs=bva_t[:, h0:h0 + 260],
                                             start=False, stop=True)
                            nc.scalar.copy(V_t[:, tb, h0:h0 + 260], pvm[:])

                # ---- qk-proj + rope for Q (m=0) and K (m=1) chunks ----
                for mloc, dest in ((0, qp_ts), (1, kp_ts)):
                    msel = 2 * p + mloc
                    mmp = ps_mm.tile([128, XC], F32, tag="mm")
                    for k in range(KT):
                        nc.tensor.matmul(
                            mmp[:], lhsT=wqk_pair[:, k, mloc * 128:(mloc + 1) * 128],
                            rhs=xc[:, k, :], start=(k == 0), stop=(k == KT - 1))
                    bcol = bqk_t[:, msel:msel + 1]
                    # T1 = (psum + b) * cos  (DVE, SBUF out)
                    t1 = t1p.tile([128, XC], F32, tag="t1")
                    nc.vector.scalar_tensor_tensor(
                        t1[:], mmp[:], bcol, cos_t[:, c0:c0 + XC],
                        op0=ALU.add, op1=ALU.mult)
                    # T2 = (psum + b) * sin  (DVE, PSUM out)
                    t2 = ps_s.tile([128, XC], F32, tag="s")
                    nc.vector.scalar_tensor_tensor(
                        t2[:], mmp[:], bcol, sin_t[:, c0:c0 + XC],
                        op0=ALU.add, op1=ALU.mult)
                    dsl = dest[c0 // QC][:, 0:XC]
                    for hh in range(2):
                        b0 = 64 * hh
                        # lo = e*c - o*s ; hi = e*s + o*c
                        nc.vector.tensor_sub(dsl[b0:b0 + 32, :],
                                             t1[b0:b0 + 32, :], t2[b0 + 32:b0 + 64, :])
                        nc.vector.tensor_add(dsl[b0 + 32:b0 + 64, :],
                                             t2[b0:b0 + 32, :], t1[b0 + 32:b0 + 64, :])

            # -------- attention for this pair --------
            at_qs = [atp.tile([128, QC], F32R, tag="attnT", name=f"at{p}_{i}") for i in range(NQC)]
            at_tiles.append(at_qs)
            for qc in range(NQC):
                q0c = qc * QC
                nkb = 4 * qc + 4
                pvA = ps_pv.tile([65, QC], F32, tag="pv")
                pvB = ps_pv.tile([65, QC], F32, tag="pv")
                s_tiles = {}

                def emit_s(kb):
                    d = kb - 4 * qc
                    v0 = 0 if d < 0 else min(128 * d, QC - 256)
                    sAB = ps_s.tile([128, 2, QC], F32, tag="s")
                    kq = kp_ts[kb // 4]
                    kc0 = (kb % 4) * 128
                    qq = qp_ts[qc]
                    nc.tensor.matmul(sAB[:, 0, v0:], lhsT=kq[0:64, kc0:kc0 + 128],
                                     rhs=qq[0:64, v0:],
                                     start=True, stop=True, tile_position=(0, 0))
                    nc.tensor.matmul(sAB[:, 1, v0:], lhsT=kq[64:128, kc0:kc0 + 128],
                                     rhs=qq[64:128, v0:],
                                     start=True, stop=True, tile_position=(64, 0))
                    s_tiles[kb] = (sAB, d, v0)

                emit_s(0)
                for kb in range(nkb):
                    if kb + 1 < nkb:
                        emit_s(kb + 1)
                    sAB, d, v0 = s_tiles.pop(kb)
                    if d == 3:
                        mb = bass.AP(mask3_t.tensor, mask3_t[:].offset,
                                     [mask3_t[:].ap[0], [0, 2], [1, 256]])
                        nc.vector.tensor_add(sAB[:, :, 256:512], sAB[:, :, 256:512], mb)
                    elif d >= 0:
                        mb = bass.AP(mask_t.tensor, mask_t[:].offset,
                                     [mask_t[:].ap[0], [0, 2], [1, 128]])
                        nc.vector.tensor_add(sAB[:, :, v0:v0 + 128],
                                             sAB[:, :, v0:v0 + 128], mb)
                    for hh, pv in ((0, pvA), (1, pvB)):
                        e = ep.tile([128, QC], F32R, tag="e")
                        nc.scalar.activation(e[:, v0:], sAB[:, hh, v0:], AF.Exp, scale=0.125)
                        nc.tensor.matmul(pv[0:65, v0:],
                                         lhsT=V_t[:, kb, (2 * p + hh) * 65:(2 * p + hh) * 65 + 65],
                                         rhs=e[:, v0:], start=(kb == 0), stop=(kb == nkb - 1))
                for hh, pv in ((0, pvA), (1, pvB)):
                    nc.vector.tensor_copy(at_qs[qc][64 * hh:64 * hh + 64, :], pv[0:64, :])
                    zrow = zbp.tile([1, QC], F32, tag="zrow")
                    nc.vector.tensor_copy(zrow[:], pv[64:65, :])
                    zb = zbp.tile([128, QC], F32, tag="zb")
                    nc.gpsimd.partition_broadcast(zb[:], zrow[:])
                    rz = zbp.tile([128, QC], F32, tag="rz")
                    nc.vector.reciprocal_approx_fast(rz[:], zb[:])
                    sl = at_qs[qc][64 * hh:64 * hh + 64, :]
                    nc.vector.tensor_mul(sl, sl, rz[64 * hh:64 * hh + 64, :])

        # -------- output projection --------
        qkv_ctx.close()
        wop = ctx.enter_context(tc.tile_pool(name="wop", bufs=1))
        outp = ctx.enter_context(tc.tile_pool(name="outp", bufs=3))
        wo_t = wop.tile([128, 4, 1024], F32R, tag="wo")
        nc.sync.dma_start(out=wo_t[:], in_=wo_d.rearrange("(k p) m -> p k m", p=128))
        for qb in range(16):
            for oc in range(2):
                po = ps_mm.tile([128, 512], F32, tag="mm")
                for p4 in range(NPAIR):
                    nc.tensor.matmul(
                        po[:], lhsT=at_tiles[p4][qb // 4][:, (qb % 4) * 128:(qb % 4) * 128 + 128],
                        rhs=wo_t[:, p4, oc * 512:(oc + 1) * 512],
                        start=(p4 == 0), stop=(p4 == NPAIR - 1))
                ot = outp.tile([128, 512], F32, tag="ot")
                nc.scalar.copy(ot[:], po[:])
                nc.sync.dma_start(out=out_d[qb * 128:(qb + 1) * 128,
                                            oc * 512:(oc + 1) * 512], in_=ot[:])


def _prep_inputs(x, W_qkv, b_qkv, W_out, cos, sin):
    """Host-side sharding/permutation. Returns list of 8 per-core in_maps."""
    x = np.ascontiguousarray(np.asarray(x, dtype=np.float32))
    W_qkv = np.asarray(W_qkv, dtype=np.float32)
    b_qkv = np.asarray(b_qkv, dtype=np.float32)
    W_out = np.asarray(W_out, dtype=np.float32)
    cos = np.asarray(cos, dtype=np.float32)
    sin = np.asarray(sin, dtype=np.float32)

    xTs = [np.ascontiguousarray(x[b].T) for b in range(B)]
    # rope tables: rows r = table[:, r % 32]
    cosT = np.ascontiguousarray(cos.T)           # [32, T]
    sinT = np.ascontiguousarray(sin.T)
    cos4 = np.ascontiguousarray(np.tile(cosT, (4, 1)))   # [128, T]
    sin4 = np.ascontiguousarray(np.tile(sinT, (4, 1)))
    ones1 = np.ones((1, 128), np.float32)

    groups = []
    for g in range(2):
        heads = [g * HPG + i for i in range(HPG)]
        qk_cols = []
        for p in range(NPAIR):
            A, Bh = heads[2 * p], heads[2 * p + 1]
            for base in (0, DK):                  # q block then k block
                for h in (A, Bh):
                    qk_cols += list(3 * DK * h + base + np.arange(0, DK, 2))
                    qk_cols += list(3 * DK * h + base + np.arange(1, DK, 2))
        qk_cols = np.array(qk_cols)
        wqk = np.ascontiguousarray(W_qkv[:, qk_cols])         # [1024, 1024]
        bqk = np.ascontiguousarray(b_qkv[qk_cols].reshape(8, 128).T)  # [128, 8]
        # v with interleaved zero cols at the ones positions: [1024, 8*65]
        wva = np.zeros((D, 520), np.float32)
        bva = np.zeros((1, 520), np.float32)
        for i, h in enumerate(heads):
            vcols = 3 * DK * h + 2 * DK + np.arange(DK)
            wva[:, i * 65:i * 65 + 64] = W_qkv[:, vcols]
            bva[0, i * 65:i * 65 + 64] = b_qkv[vcols]
            bva[0, i * 65 + 64] = 1.0                 # ones column
        wo = np.ascontiguousarray(W_out[g * 512:(g + 1) * 512, :])
        groups.append(dict(wqk=wqk, bqk=bqk, wva=np.ascontiguousarray(wva),
                           bva=bva, wo=wo))

    in_maps = []
    for c in range(NC_):
        b, g = c // 2, c % 2
        gr = groups[g]
        in_maps.append({
            "xT": xTs[b], "wqk": gr["wqk"], "wva": gr["wva"], "bva": gr["bva"],
            "ones1": ones1, "wo": gr["wo"], "bqk": gr["bqk"],
            "cos4": cos4, "sin4": sin4,
        })
    return in_maps


def run(x, W_qkv, b_qkv, W_out, b_out, cos, sin, trace=False, trace_cores=None):
    """Build/compile (cached), run on 8 cores, return (out, BassKernelResults)."""
    if "nc" not in _cache:
        _cache["nc"] = _build_nc()
    nc = _cache["nc"]
    in_maps = _prep_inputs(x, W_qkv, b_qkv, W_out, cos, sin)
    kw = {}
    if trace:
        kw = dict(trace=True, trace_cores=trace_cores or [0])
    res = bass_utils.run_bass_kernel_spmd(nc, in_maps, core_ids=list(range(NC_)), **kw)
    b_out = np.asarray(b_out, dtype=np.float32)
    out = np.empty((B, T, D), np.float32)
    for b in range(B):
        out[b] = res.results[2 * b]["out"] + res.results[2 * b + 1]["out"] + b_out[None, :]
    return out, res


def kernel(x, W_qkv, b_qkv, W_out, b_out, cos, sin):
    out, _ = run(x, W_qkv, b_qkv, W_out, b_out, cos, sin)
    return out



# revision 24
# speedup vs baseline: 1.5154x; 1.5154x over previous
"""Causal self-attention (B=4, T=2048, D=1024, H=16) on 8 TRN2 NeuronCores.

Sharding: core c handles batch b=c//2 and head-group g=c%2 (8 heads).
Each core computes its heads' attention + a partial output projection
(contraction over its 512 attn channels); the host sums the two partials
per batch and adds b_out.

v2: bf16 matmul operands throughout (fp32 PSUM accumulation), x/weights
preloaded to SBUF once, merged two-head exp per key block, exact causal
widths on diagonal blocks, PSUM-fused softmax normalization, doubled qk
tile pool so pair p+1's projection+rope overlaps pair p's attention.

Per-core device kernel (channels-on-partitions layout):
  qk-proj   qkT[ch,T] = wqk.T @ xT          (ch-major, per head-pair m-chunks)
  v-proj    V[t,ch']  = xT.T @ wv_aug        (65-wide per head: 64 v cols + a
                                              ones col for the softmax
                                              normalizer; bias via ones-row mm)
  rope      q',k' via DVE STT + quarter sub/adds, bf16 out
  S^T       [k,q] = k'^T q' per head, 2 heads packed via tile_position
  softmax   no-max-subtraction exp (score range ~|8|), causal tri mask added
            in PSUM on diagonal 128-blocks, one [128,2,w] exp per key block,
            normalizer from the V ones col
  PV        attn_aug^T[65,q] = V_aug^T @ E^T accumulated over k blocks
  norm      1/Z via DVE recip from PSUM row, gpsimd partition_broadcast,
            fused (PSUM * zb -> bf16 SBUF) multiply
  out-proj  out[q,o] = attnT.T @ wo  (partial; host adds pair partials)
"""
import sys
import numpy as np

for _p in ("/opt/trn_rl_repo", "/root/.axon_site/_ro/trn_rl_repo"):
    if _p not in sys.path:
        sys.path.append(_p)

import ml_dtypes
import concourse.bass as bass
import concourse.bacc as bacc
import concourse.tile as tile
import concourse.mybir as mybir
from concourse import bass_utils

F32 = mybir.dt.float32
BF16 = mybir.dt.bfloat16
AF = mybir.ActivationFunctionType
ALU = mybir.AluOpType
BF = ml_dtypes.bfloat16

B, T, D, H, DK = 4, 2048, 1024, 16, 64
NC_ = 8          # cores
HPG = 8          # heads per group
NPAIR = 4        # head pairs per core
KT = 8           # 128-row k-tiles over D
XC = 512         # x/qkv t-chunk width
NXC = T // XC    # 4
QC = 512         # attention q-chunk width
NQC = T // QC    # 4
NKB = T // 128   # 16 key blocks
MASK_VAL = -30000.0
# stream_shuffle permutes within each 32-partition block (mask replicated
# across the four blocks): rotate by 16 to swap the e/o halves of a block
SWAP16_MASK = [(i + 16) % 32 for i in range(32)]

_cache = {}


def _build_nc(trace_scopes=False):
    nc = bacc.Bacc("TRN2", target_bir_lowering=False, debug=False)

    xT_d = nc.dram_tensor("xT", [D, T], BF16, kind="ExternalInput").ap()
    wqk_d = nc.dram_tensor("wqk", [D, 1024], BF16, kind="ExternalInput").ap()
    wva_d = nc.dram_tensor("wva", [D, 520], BF16, kind="ExternalInput").ap()
    bva_d = nc.dram_tensor("bva", [1, 520], BF16, kind="ExternalInput").ap()
    ones_d = nc.dram_tensor("ones1", [1, 128], BF16, kind="ExternalInput").ap()
    wo_d = nc.dram_tensor("wo", [512, 1024], BF16, kind="ExternalInput").ap()
    bqk_d = nc.dram_tensor("bqk", [128, 8], F32, kind="ExternalInput").ap()
    bqksw_d = nc.dram_tensor("bqksw", [128, 8], F32, kind="ExternalInput").ap()
    cos_d = nc.dram_tensor("cos4", [128, T], F32, kind="ExternalInput").ap()
    sin_d = nc.dram_tensor("sin4", [128, T], BF16, kind="ExternalInput").ap()
    out_d = nc.dram_tensor("out", [T, 1024], F32, kind="ExternalOutput").ap()

    with tile.TileContext(nc, pool_alloc_mode="queue") as tc:
        _emit(tc, nc, xT_d, wqk_d, wva_d, bva_d, ones_d, wo_d, bqk_d,
              bqksw_d, cos_d, sin_d, out_d)
    nc.compile()
    return nc


def _emit(tc, nc, xT_d, wqk_d, wva_d, bva_d, ones_d, wo_d, bqk_d,
          bqksw_d, cos_d, sin_d, out_d,
          dbg_qk_d=None, dbg_at_d=None, dbg_v_d=None, dbg_zb_d=None):
    from contextlib import ExitStack
    ctx = ExitStack()
    with ctx:
        consts = ctx.enter_context(tc.tile_pool(name="consts", bufs=1))
        vpool = ctx.enter_context(tc.tile_pool(name="vpool", bufs=1))
        qkp = ctx.enter_context(tc.tile_pool(name="qkp", bufs=8))
        t1p = ctx.enter_context(tc.tile_pool(name="t1p", bufs=3))
        ep = ctx.enter_context(tc.tile_pool(name="ep", bufs=4))
        zbp = ctx.enter_context(tc.tile_pool(name="zbp", bufs=2))
        atp = ctx.enter_context(tc.tile_pool(name="atp", bufs=16))
        wop = ctx.enter_context(tc.tile_pool(name="wop", bufs=1))
        outp = ctx.enter_context(tc.tile_pool(name="outp", bufs=4))
        ps_mm = ctx.enter_context(tc.tile_pool(name="ps_mm", bufs=2, space="PSUM"))
        ps_s = ctx.enter_context(tc.tile_pool(name="ps_s", bufs=2, space="PSUM"))
        ps_pv = ctx.enter_context(tc.tile_pool(name="ps_pv", bufs=2, space="PSUM"))

        # ---------------- constants ----------------
        cos_t = consts.tile([128, T], F32, tag="cos")
        nc.sync.dma_start(out=cos_t[:], in_=cos_d)
        sin_t = consts.tile([128, T], BF16, tag="sin")
        nc.sync.dma_start(out=sin_t[:], in_=sin_d)
        bqk_t = consts.tile([128, 8], F32, tag="bqk")
        nc.sync.dma_start(out=bqk_t[:], in_=bqk_d)
        bqksw_t = consts.tile([128, 8], F32, tag="bqksw")
        nc.sync.dma_start(out=bqksw_t[:], in_=bqksw_d)
        bva_t = consts.tile([1, 520], BF16, tag="bva")
        nc.sync.dma_start(out=bva_t[:], in_=bva_d)
        ones_t = consts.tile([1, 128], BF16, tag="ones")
        nc.sync.dma_start(out=ones_t[:], in_=ones_d)
        wva_t = consts.tile([128, KT, 520], BF16, tag="wva")
        nc.sync.dma_start(out=wva_t[:], in_=wva_d.rearrange("(k p) m -> p k m", p=128))
        # additive causal tri mask [128,128]: keep where col-row>=0
        mask_t = consts.tile([128, 128], F32, tag="mask")
        nc.gpsimd.memset(mask_t[:], 0.0)
        nc.gpsimd.affine_select(
            out=mask_t[:], in_=mask_t[:], compare_op=ALU.is_ge, fill=MASK_VAL,
            base=0, pattern=[[1, 128]], channel_multiplier=-1)

        # x and wqk preloaded once (bf16)
        xT_r = xT_d.rearrange("(k p) t -> p k t", p=128)
        x_ts = []
        for tq in range(NXC):
            xt = consts.tile([128, KT, XC], BF16, tag=f"xT{tq}", name=f"xT{tq}")
            nc.sync.dma_start(out=xt[:], in_=xT_r[:, :, tq * XC:(tq + 1) * XC])
            x_ts.append(xt)
        wqk_r = wqk_d.rearrange("(k p) m -> p k m", p=128)
        wqk_ts = []
        for p in range(NPAIR):
            wt = consts.tile([128, KT, 256], BF16, tag=f"wqk{p}", name=f"wqk{p}")
            nc.sync.dma_start(out=wt[:], in_=wqk_r[:, :, 256 * p:256 * (p + 1)])
            wqk_ts.append(wt)

        # V_aug for all 16 t-blocks: [128 tok, 16 * (8 heads * 65)]
        V_t = vpool.tile([128, NKB, 520], BF16, tag="V")
        wo_t = wop.tile([128, 4, 1024], BF16, tag="wo")
        nc.sync.dma_start(out=wo_t[:], in_=wo_d.rearrange("(k p) m -> p k m", p=128))

        at_tiles = []
        for p in range(NPAIR):
            wqk_pair = wqk_ts[p]
            qp_ts = [qkp.tile([128, QC], BF16, tag="qp", name=f"qp{p}_{i}") for i in range(NQC)]
            kp_ts = [qkp.tile([128, QC], BF16, tag="kp", name=f"kp{p}_{i}") for i in range(NQC)]

            for tq in range(NXC):
                c0 = tq * XC
                xc = x_ts[tq]

                if p == 0:
                    # ---- v-proj for the 4 t-blocks in this chunk ----
                    for tb2 in range(XC // 128):
                        tb = tq * (XC // 128) + tb2
                        for half in range(2):
                            h0 = half * 260
                            pvm = ps_mm.tile([128, 260], F32, tag="mm")
                            for k in range(KT):
                                nc.tensor.matmul(
                                    pvm[:], lhsT=xc[:, k, tb2 * 128:(tb2 + 1) * 128],
                                    rhs=wva_t[:, k, h0:h0 + 260],
                                    start=(k == 0), stop=False)
                            nc.tensor.matmul(pvm[:], lhsT=ones_t[:],
                                             rhs=bva_t[:, h0:h0 + 260],
                                             start=False, stop=True)
                            nc.scalar.copy(V_t[:, tb, h0:h0 + 260], pvm[:])

                # ---- qk-proj + rope for Q (m=0) and K (m=1) chunks ----
                # rope via partition-swap: mms = swap32(psum) (stream_shuffle),
                # t1 = (psum+b)*cos, t2s = (mms+b_sw)*sin_signed (4x bf16 STT,
                # sign folded into the host table), q' = t1 + t2s (one TT add).
                for mloc, dest in ((0, qp_ts), (1, kp_ts)):
                    msel = 2 * p + mloc
                    mmp = ps_mm.tile([128, XC], F32, tag="mm")
                    for k in range(KT):
                        nc.tensor.matmul(
                            mmp[:], lhsT=wqk_pair[:, k, mloc * 128:(mloc + 1) * 128],
                            rhs=xc[:, k, :], start=(k == 0), stop=(k == KT - 1))
                    bcol = bqk_t[:, msel:msel + 1]
                    bcol_sw = bqksw_t[:, msel:msel + 1]
                    mms = t1p.tile([128, XC], F32, tag="mms")
                    nc.vector.stream_shuffle(mms[:], mmp[:], mask=SWAP16_MASK)
                    t1 = t1p.tile([128, XC], BF16, tag="t1")
                    nc.vector.scalar_tensor_tensor(
                        t1[:], mmp[:], bcol, cos_t[:, c0:c0 + XC],
                        op0=ALU.add, op1=ALU.mult)
                    t2s = t1p.tile([128, XC], BF16, tag="t2s")
                    nc.vector.scalar_tensor_tensor(
                        t2s[:], mms[:], bcol_sw, sin_t[:, c0:c0 + XC],
                        op0=ALU.add, op1=ALU.mult)
                    dsl = dest[c0 // QC][:, 0:XC]
                    nc.vector.tensor_add(dsl[:, :], t1[:], t2s[:])

            if p == 0 and dbg_qk_d is not None:
                dbgt = t1p.tile([128, QC], F32, tag="t1", name="dbgqk")
                for i in range(NQC):
                    nc.vector.tensor_copy(dbgt[:], qp_ts[i][:])
                    nc.sync.dma_start(out=dbg_qk_d[:, i * QC:(i + 1) * QC], in_=dbgt[:])
                    nc.vector.tensor_copy(dbgt[:], kp_ts[i][:])
                    nc.sync.dma_start(out=dbg_qk_d[:, (4 + i) * QC:(4 + i + 1) * QC], in_=dbgt[:])

            # -------- attention for this pair --------
            at_qs = [atp.tile([128, QC], BF16, tag="attnT", name=f"at{p}_{i}") for i in range(NQC)]
            at_tiles.append(at_qs)
            for qc in range(NQC):
                nkb = 4 * qc + 4
                pvA = ps_pv.tile([65, QC], F32, tag="pv")
                pvB = ps_pv.tile([65, QC], F32, tag="pv")
                s_tiles = {}

                def emit_s(kb):
                    d = kb - 4 * qc
                    v0 = 0 if d < 0 else 128 * d
                    sAB = ps_s.tile([128, 2, QC], F32, tag="s")
                    kq = kp_ts[kb // 4]
                    kc0 = (kb % 4) * 128
                    qq = qp_ts[qc]
                    nc.tensor.matmul(sAB[:, 0, v0:], lhsT=kq[0:64, kc0:kc0 + 128],
                                     rhs=qq[0:64, v0:],
                                     start=True, stop=True, tile_position=(0, 0))
                    nc.tensor.matmul(sAB[:, 1, v0:], lhsT=kq[64:128, kc0:kc0 + 128],
                                     rhs=qq[64:128, v0:],
                                     start=True, stop=True, tile_position=(64, 0))
                    s_tiles[kb] = (sAB, d, v0)

                emit_s(0)
                for kb in range(nkb):
                    if kb + 1 < nkb:
                        emit_s(kb + 1)
                    sAB, d, v0 = s_tiles.pop(kb)
                    if d >= 0:
                        # causal tri mask on the diagonal 128-block (both heads)
                        mb = bass.AP(mask_t.tensor, mask_t[:].offset,
                                     [mask_t[:].ap[0], [0, 2], [1, 128]])
                        nc.vector.tensor_add(sAB[:, :, v0:v0 + 128],
                                             sAB[:, :, v0:v0 + 128], mb)
                    # one exp for both heads
                    e = ep.tile([128, 2, QC], BF16, tag="e")
                    nc.scalar.activation(e[:, :, v0:], sAB[:, :, v0:], AF.Exp,
                                         scale=0.125)
                    for hh, pv in ((0, pvA), (1, pvB)):
                        nc.tensor.matmul(pv[0:65, v0:],
                                         lhsT=V_t[:, kb, (2 * p + hh) * 65:(2 * p + hh) * 65 + 65],
                                         rhs=e[:, hh, v0:], start=(kb == 0), stop=(kb == nkb - 1))
                # normalization: at[h] = pv[0:64] * bcast(1/Z)
                zzA = zbp.tile([1, QC], F32, tag="zzA")
                nc.vector.tensor_copy(zzA[:], pvA[64:65, :])
                zzB = zbp.tile([1, QC], F32, tag="zzB")
                nc.vector.tensor_copy(zzB[:], pvB[64:65, :])
                rzA = zbp.tile([1, QC], F32, tag="rzA")
                nc.vector.reciprocal_approx_fast(rzA[:], zzA[:])
                rzB = zbp.tile([1, QC], F32, tag="rzB")
                nc.vector.reciprocal_approx_fast(rzB[:], zzB[:])
                # partition_broadcast only honors offset-0 dests on HW:
                # broadcast each head's 1/Z to its own offset-0 tile
                zbA = zbp.tile([64, QC], F32, tag="zbA")
                nc.gpsimd.partition_broadcast(zbA[:], rzA[:])
                zbB = zbp.tile([128, QC], F32, tag="zbB")
                nc.gpsimd.partition_broadcast(zbB[:], rzB[:])
                at = at_qs[qc]
                nc.vector.tensor_mul(at[0:64, :], pvA[0:64, :], zbA[:])
                nc.vector.tensor_copy(at[64:128, :], pvB[0:64, :])
                nc.vector.tensor_mul(at[64:128, :], at[64:128, :], zbB[64:128, :])

        if dbg_v_d is not None:
            dvt = t1p.tile([128, 520], F32, tag="t1", name="dbgv")
            for kb in range(NKB):
                nc.vector.tensor_copy(dvt[:], V_t[:, kb, :])
                nc.sync.dma_start(out=dbg_v_d[:, kb * 520:(kb + 1) * 520], in_=dvt[:])
        if dbg_at_d is not None:
            dat = t1p.tile([128, QC], F32, tag="t1", name="dbgat")
            for p4 in range(NPAIR):
                for i in range(NQC):
                    nc.vector.tensor_copy(dat[:], at_tiles[p4][i][:])
                    nc.sync.dma_start(
                        out=dbg_at_d[:, (p4 * 4 + i) * QC:(p4 * 4 + i + 1) * QC],
                        in_=dat[:])

        # -------- output projection --------
        # PSUM comes from the still-open attention pools (same-pool aliasing
        # is dependency-tracked; a fresh pool over the same banks is not)
        for qb in range(16):
            if qb % 3 == 0:
                sp = ps_s.tile([128, 2, QC], F32, tag="s")
                poA, poB = sp[:, 0, :], sp[:, 1, :]
            elif qb % 3 == 1:
                poAt = ps_mm.tile([128, 512], F32, tag="mm", name=f"poA{qb}")
                poBt = ps_mm.tile([128, 512], F32, tag="mm", name=f"poB{qb}")
                poA, poB = poAt[:], poBt[:]
            else:
                poAt = ps_pv.tile([128, 512], F32, tag="pv", name=f"poA{qb}")
                poBt = ps_pv.tile([128, 512], F32, tag="pv", name=f"poB{qb}")
                poA, poB = poAt[:], poBt[:]
            for p4 in range(NPAIR):
                lt = at_tiles[p4][qb // 4][:, (qb % 4) * 128:(qb % 4) * 128 + 128]
                nc.tensor.matmul(poA, lhsT=lt, rhs=wo_t[:, p4, 0:512],
                                 start=(p4 == 0), stop=(p4 == NPAIR - 1))
                nc.tensor.matmul(poB, lhsT=lt, rhs=wo_t[:, p4, 512:1024],
                                 start=(p4 == 0), stop=(p4 == NPAIR - 1))
            for oc, po in ((0, poA), (1, poB)):
                ot = outp.tile([128, 512], F32, tag="ot")
                nc.scalar.copy(ot[:], po)
                nc.sync.dma_start(out=out_d[qb * 128:(qb + 1) * 128,
                                            oc * 512:(oc + 1) * 512], in_=ot[:])


def _prep_inputs(x, W_qkv, b_qkv, W_out, cos, sin):
    """Host-side sharding/permutation. Returns list of 8 per-core in_maps."""
    x = np.ascontiguousarray(np.asarray(x, dtype=np.float32))
    W_qkv = np.asarray(W_qkv, dtype=np.float32)
    b_qkv = np.asarray(b_qkv, dtype=np.float32)
    W_out = np.asarray(W_out, dtype=np.float32)
    cos = np.asarray(cos, dtype=np.float32)
    sin = np.asarray(sin, dtype=np.float32)

    xTs = [np.ascontiguousarray(x[b].T.astype(BF)) for b in range(B)]
    # rope tables for row layout r -> rotary index i = 16*(r%64//32) + r%16;
    # rows with (r%32)<16 hold the e-half (lo out: e*cos - o*sin), rows with
    # (r%32)>=16 hold the o-half (hi out: o*cos + e*sin). sin sign folded in.
    r = np.arange(128)
    ri = 16 * ((r % 64) // 32) + (r % 16)            # rotary pair index
    sgn = np.where((r % 32) < 16, -1.0, 1.0).astype(np.float32)
    cos4 = np.ascontiguousarray(cos.T[ri])           # [128, T]
    sin4 = np.ascontiguousarray((sin.T[ri] * sgn[:, None]).astype(BF))
    ones1 = np.ones((1, 128), BF)

    groups = []
    for g in range(2):
        heads = [g * HPG + i for i in range(HPG)]
        qk_cols = []
        for p in range(NPAIR):
            A, Bh = heads[2 * p], heads[2 * p + 1]
            for base in (0, DK):                  # q block then k block
                for h in (A, Bh):
                    for blk in range(2):          # [e0..15, o0..15] per 32-blk
                        ii = 16 * blk + np.arange(16)
                        qk_cols += list(3 * DK * h + base + 2 * ii)
                        qk_cols += list(3 * DK * h + base + 2 * ii + 1)
        qk_cols = np.array(qk_cols)
        wqk = np.ascontiguousarray(W_qkv[:, qk_cols].astype(BF))      # [1024, 1024]
        bqk = np.ascontiguousarray(b_qkv[qk_cols].reshape(8, 128).T)  # [128, 8] f32
        # bias rows in swapped order for the shuffled STT operand
        rr = np.arange(128)
        sw = (rr // 32) * 32 + (rr + 16) % 32
        bqksw = np.ascontiguousarray(bqk[sw])
        # v with interleaved normalizer cols: [1024, 8*65]
        wva = np.zeros((D, 520), np.float32)
        bva = np.zeros((1, 520), np.float32)
        for i, h in enumerate(heads):
            vcols = 3 * DK * h + 2 * DK + np.arange(DK)
            wva[:, i * 65:i * 65 + 64] = W_qkv[:, vcols]
            bva[0, i * 65:i * 65 + 64] = b_qkv[vcols]
            bva[0, i * 65 + 64] = 1.0                 # ones column
        wo = np.ascontiguousarray(W_out[g * 512:(g + 1) * 512, :].astype(BF))
        groups.append(dict(wqk=wqk, bqk=bqk, bqksw=bqksw,
                           wva=np.ascontiguousarray(wva.astype(BF)),
                           bva=bva.astype(BF), wo=wo))

    in_maps = []
    for c in range(NC_):
        b, g = c // 2, c % 2
        gr = groups[g]
        in_maps.append({
            "xT": xTs[b], "wqk": gr["wqk"], "wva": gr["wva"], "bva": gr["bva"],
            "ones1": ones1, "wo": gr["wo"], "bqk": gr["bqk"],
            "bqksw": gr["bqksw"], "cos4": cos4, "sin4": sin4,
        })
    return in_maps


def run(x, W_qkv, b_qkv, W_out, b_out, cos, sin, trace=False, trace_cores=None):
    """Build/compile (cached), run on 8 cores, return (out, BassKernelResults)."""
    if "nc" not in _cache:
        _cache["nc"] = _build_nc()
    nc = _cache["nc"]
    in_maps = _prep_inputs(x, W_qkv, b_qkv, W_out, cos, sin)
    kw = {}
    if trace:
        kw = dict(trace=True, trace_cores=trace_cores or [0])
    res = bass_utils.run_bass_kernel_spmd(nc, in_maps, core_ids=list(range(NC_)), **kw)
    b_out = np.asarray(b_out, dtype=np.float32)
    out = np.empty((B, T, D), np.float32)
    for b in range(B):
        out[b] = res.results[2 * b]["out"] + res.results[2 * b + 1]["out"] + b_out[None, :]
    return out, res


def kernel(x, W_qkv, b_qkv, W_out, b_out, cos, sin):
    out, _ = run(x, W_qkv, b_qkv, W_out, b_out, cos, sin)
    return out
